# revision 11
# baseline (speedup 1.0000x reference)
"""Single-head attention (qkv-proj + softmax(QK^T)V) on 8 TRN2 NeuronCores.

Sharding: batch (4) x query-half (2) -> 8 shards. Each core computes full
k/v for its batch (duplicated across the 2 cores sharing a batch) and
attention for its 2048 query rows. For odd cores the host rotates the
sequence axis of x^T so the core's own query half occupies columns 0:2048;
k/v ordering over s is irrelevant (softmax sum + AV contraction are
permutation-invariant when k and v share the ordering).

Per-core device kernel (bf16 matmuls, fp32 PSUM accumulation). Work is
emitted as (s-tile, ch) UNITS of 1024 query columns each (ch = q-column
half): 2 scores matmuls -> one [128,1024] PSUM tile -> one Exp activation
(FD=1024, scale fused, no max subtraction -- scores are bounded ~8) -> 2
AV matmuls accumulating o_t[ch] [128,1024] PSUM over all 32 s-tiles. AV
for a unit is emitted 4 units late (pend depth 4) so TensorE never waits
on ScalarE's exp stream.

The unit split (vs whole 2048-col s-tiles) exists for the FRONT of the
pipeline: ch0 units need only q chunks 0-1, so the scores/exp stream
starts as soon as x cols 0:1024 have landed (~17us) instead of waiting
for all of qT (~25us). Unit order: (s=0..5, ch0), (s=0..5, ch1), then
(s, ch0),(s, ch1) for s=6..31. Projection jobs (8 accumulation matmuls +
DVE bias-copy each) fill the PE between units, deadline-placed with 2-3
s-tiles of slack against their consumers and the x^T DMA wave arrivals.

HAM warmup starts at ~6.8us off a GpSimd-memset tile (GpSimd's queue is
free at kernel start; DVE's isn't) so the PE clock gate is 8/8 and the
Exp ACT_TABLE is loaded before the first projection matmul issues.

Softmax denominators: VectorE ping-pong-accumulates acc[ch] += exp unit
(bf16 2x mode); the 128-partition reduction and the divide run on the
host in float64. The last unit per ch ships its exp tile raw (e31); the
host folds its column sums into the denominator. Outputs outT/acc/e31
drain per-ch so ch0's copies+DMAs overlap ch1's final AV matmuls.

PSUM budget: scores pool 2x[128,1024] (4 banks, shared with projection
jobs' [128,512] tiles) + two o_t accumulators (4 banks) = 8 banks.
"""

import numpy as np
import ml_dtypes

import concourse.bass as bass
import concourse.tile as tile
from concourse import bacc, mybir
from concourse import bass_utils

BF16 = ml_dtypes.bfloat16
F32 = mybir.dt.float32
BF = mybir.dt.bfloat16
AF = mybir.ActivationFunctionType

B = 4
T = 4096
DMODEL = 1024
DIM = 128
NCORES = 8
THALF = T // 2          # 2048 query rows per core
NDIN = DMODEL // 128    # 8 contraction tiles
NS = T // 128           # 32 key/value s-tiles
SCALE = float(DIM) ** -0.5

_nc_cache = []


def _emit(nc, tc, ap):
    P = 128
    from contextlib import ExitStack
    with ExitStack() as ctx:
        res = ctx.enter_context(tc.tile_pool(name="resident", bufs=1))

        # ---- batched input DMAs (few, large, multi-dim-AP transfers,
        # need-ordered on ONE HWDGE ring: the SDMA engines round-robin
        # between rings at packet granularity, so a second ring would
        # steal bandwidth from the first-needed transfer) ----
        xw = []
        woff = []

        def load_wave(cc, o, w, din0=0, ndin=NDIN):
            t_ = res.tile([P, ndin, w], BF, tag=f"xw{cc}", name=f"xw{cc}")
            src = ap["xT"].ap()[din0 * P:(din0 + ndin) * P, o:o + w] \
                .rearrange("(n p) w -> p n w", p=P)
            nc.sync.dma_start(t_[:], src)
            xw.append(t_)
            woff.append((o, w, din0, ndin))

        # weights in three per-tensor pieces so each lands just before its
        # first projection; wave 0 split by contraction half so the first
        # k projection can start on dins 0-3 while 4-7 are still arriving
        wtile = {}
        for nm in ("wk", "wq", "wv"):
            wtile[nm] = res.tile([P, NDIN * P + 1], BF, tag=nm, name=nm)
        nc.sync.dma_start(wtile["wk"][:], ap["wk"].ap())
        load_wave("0a", 0, 512, din0=0, ndin=4)
        load_wave("0b", 0, 512, din0=4, ndin=4)
        nc.sync.dma_start(wtile["wq"][:], ap["wq"].ap())
        load_wave("1", 512, 512)
        nc.sync.dma_start(wtile["wv"][:], ap["wv"].ap())
        load_wave("2", 1024, 1024)
        load_wave("3", 2048, 1024)
        load_wave("4", 3072, 1024)

        w_sb = {nm: wtile[nm][:, 0:NDIN * P].rearrange(
            "p (n e) -> p n e", n=NDIN) for nm in wtile}
        bias_f = res.tile([P, 3], F32, tag="bias_f")
        for i, nm in enumerate(("wq", "wv", "wk")):
            nc.vector.tensor_copy(bias_f[:, i:i + 1],
                                  wtile[nm][:, NDIN * P:NDIN * P + 1])
        bias = {"bq": bias_f[:, 0:1], "bv": bias_f[:, 1:2],
                "bk": bias_f[:, 2:3]}

        def xchunk(d, c):
            """x^T [128, 512] slice for 512-col chunk c, din tile d."""
            o = c * 512
            for cc, (wo, w, din0, ndin) in enumerate(woff):
                if wo <= o < wo + w and din0 <= d < din0 + ndin:
                    return xw[cc][:, d - din0, o - wo:o - wo + 512]
            raise AssertionError

        kT = res.tile([P, T], BF, tag="kT")
        vT = res.tile([P, T], BF, tag="vT")
        qT = res.tile([P, THALF], BF, tag="qT")
        v_sb = res.tile([P, T], BF, tag="v_sb")
        # per-ch ping-pong accumulators for the softmax denominators
        accs = [[res.tile([P, 1024], BF, tag=f"acc{ch}{i}",
                          name=f"acc{ch}{i}") for i in range(2)]
                for ch in range(2)]
        outT_sb = res.tile([P, THALF], BF, tag="outT_sb")

        sc_ps = ctx.enter_context(
            tc.tile_pool(name="sc_ps", bufs=2, space="PSUM"))
        o_ps = ctx.enter_context(
            tc.tile_pool(name="o_ps", bufs=2, space="PSUM"))
        e_sb = ctx.enter_context(tc.tile_pool(name="e_sb", bufs=8))

        # HAM warmup: spin matmuls on a GpSimd-zeroed tile from ~6.8us
        # (GpSimd's queue is free right after the framework preamble;
        # DVE's memset would only land at ~7.8us). The 4096-cycle HAM
        # window then flips to 8/8 right as the first x^T wave lands, so
        # the real projections run at full clock from their first MM.
        warm = res.tile([P, 512], BF, tag="warm")
        nc.gpsimd.memset(warm[:], 0.0)
        # dummy 1-element Exp pulls the ~2.7us ACT_TABLE_LOAD into the
        # idle head window instead of stalling the first real exp
        ewarm = res.tile([P, 1], BF, tag="ewarm")
        nc.scalar.activation(ewarm[:], warm[:, 0:1], AF.Exp,
                             bias=0.0, scale=1.0)
        # one accumulation group -> no inter-matmul semaphores, so the PE
        # actually runs back-to-back and the HAM activity window fills
        wps = sc_ps.tile([P, 512], F32, tag="sc", name="wps")
        NWARM = 11
        for i in range(NWARM):
            nc.tensor.matmul(wps[:], warm[:, 0:P], warm[:],
                             start=(i == 0), stop=(i == NWARM - 1))

        def proj_job(c, dst, wnm, bnm):
            """One projection job: 512 cols of dst via 8 accumulating
            matmuls (PSUM tile borrowed from the scores pool) + DVE
            bias-add copy."""
            p = sc_ps.tile([P, 512], F32, tag="sc", name="pj")
            for din in range(NDIN):
                nc.tensor.matmul(
                    p[:], w_sb[wnm][:, din], xchunk(din, c),
                    start=(din == 0), stop=(din == NDIN - 1))
            nc.vector.tensor_scalar_add(
                dst[:, c * 512:(c + 1) * 512], p[:], bias[bnm])

        def v_transposes(c):
            # v natural tiles via DMA xbar transposes on the sync ring
            for s in range(c * 4, (c + 1) * 4):
                nc.sync.dma_start_transpose(
                    v_sb[:, s * P:(s + 1) * P], vT[:, s * P:(s + 1) * P])

        o_t = [o_ps.tile([P, 1024], F32, tag="o", name=f"o_t{i}")
               for i in range(2)]
        pend = []
        nacc = [0, 0]

        def flush_unit():
            e, s, ch = pend.pop(0)
            vs = v_sb[:, s * P:(s + 1) * P]
            st, sp = (s == 0), (s == NS - 1)
            nc.tensor.matmul(o_t[ch][:, 0:512], vs, e[:, 0:512],
                             start=st, stop=sp)
            nc.tensor.matmul(o_t[ch][:, 512:1024], vs, e[:, 512:1024],
                             start=st, stop=sp)
            if s == NS - 1:
                # last exp unit ships raw; the host folds its column sums
                # into the denominator
                nc.sync.dma_start(
                    ap["e31"].ap()[:, ch * 1024:(ch + 1) * 1024], e[:])
                # o_t[ch] is now complete: drain it (copies split across
                # DVE and ScalarE; ch0's drain overlaps ch1's last AVs)
                for j in range(2):
                    lo = ch * 1024 + j * 512
                    piece = outT_sb[:, lo:lo + 512]
                    src = o_t[ch][:, j * 512:j * 512 + 512]
                    if j == 0:
                        nc.vector.tensor_copy(piece, src)
                    else:
                        nc.scalar.copy(piece, src)
                    nc.sync.dma_start(ap["outT"].ap()[:, lo:lo + 512], piece)
                return
            n = nacc[ch]
            dst = accs[ch][n % 2][:]
            if n == 0:
                nc.vector.tensor_copy(dst, e[:])
            else:
                nc.vector.tensor_add(dst, accs[ch][(n - 1) % 2][:], e[:])
            nacc[ch] = n + 1
            if s == NS - 2:
                # acc[ch] is final (sums s=0..30); its DMA is queued before
                # the e31/outT DMAs of this ch and the ring is FIFO
                nc.sync.dma_start(
                    ap["acc"].ap()[:, ch * 1024:(ch + 1) * 1024], dst)

        def attn_unit(s, ch, jobs=(), tr=None, spacer=False):
            # PE order: AV flush first (its exp landed 4 units ago --
            # guaranteed-ready work that absorbs the PSUM-slot recycle
            # wait of the projection job), then the projection (so its
            # DVE bias-add overlaps the scores matmuls instead of
            # stalling the NEXT unit's projection), then scores.
            if len(pend) >= 4:
                flush_unit()
            elif spacer:
                # pre-flush units have no ready AV work; burn two dummy
                # matmuls into o_t[1] (whose first real AV, start=True,
                # clears the bank) so the PE stays busy across the
                # previous projection's DVE bias-add turnaround
                for _ in range(2):
                    nc.tensor.matmul(o_t[1][:, 0:512], warm[:, 0:P],
                                     warm[:], start=True, stop=True)
            for job in jobs:
                proj_job(*job)
            ks = kT[:, s * P:(s + 1) * P]
            sc = sc_ps.tile([P, 1024], F32, tag="sc", name=f"sc{s}_{ch}")
            q0 = ch * 1024
            nc.tensor.matmul(sc[:, 0:512], ks, qT[:, q0:q0 + 512],
                             start=True, stop=True)
            nc.tensor.matmul(sc[:, 512:1024], ks, qT[:, q0 + 512:q0 + 1024],
                             start=True, stop=True)
            if tr is not None:
                v_transposes(tr)
            e = e_sb.tile([P, 1024], BF, tag="e", name=f"e{s}_{ch}")
            nc.scalar.activation(e[:], sc[:], AF.Exp, bias=0.0, scale=SCALE)
            pend.append((e, s, ch))

        # ---- emission ----
        # Up-front projections: k c0 (kT tiles 0-3), q c0, q c1 -- the
        # minimum for unit (0, ch0) -- plus v c0 for the first AV flushes.
        proj_job(0, kT, "wk", "bk")
        proj_job(0, qT, "wq", "bq")
        proj_job(1, qT, "wq", "bq")
        proj_job(0, vT, "wv", "bv")
        v_transposes(0)

        jk = lambda c: (c, kT, "wk", "bk")
        jv = lambda c: (c, vT, "wv", "bv")
        jq = lambda c: (c, qT, "wq", "bq")

        # Fillers spread ~uniformly (one job per 2-3 units) so the PE
        # stays ahead of the 1.1us/unit exp stream all the way to the
        # tail; each job still lands 2+ s-tiles before its consumer
        # (k c by scores(4c, ch0), v c by the transpose at s=4c-2) and
        # after its x^T wave.
        # phase 1: (s=0..5, ch0); q c2/c3 land before phase 2 needs them.
        p1_fill = {0: [jk(1)], 1: [jv(1)], 2: [jq(2)], 3: [jq(3)],
                   4: [jk(2)], 5: [jv(2)]}
        p1_tr = {2: 1}
        for s in range(6):
            attn_unit(s, 0, p1_fill.get(s, ()), p1_tr.get(s),
                      spacer=(s in (1, 2, 3)))
        # phase 2: (s=0..5, ch1)
        p2_fill = {0: [jk(3)], 2: [jv(3)]}
        p2_tr = {0: 2}
        for s in range(6):
            attn_unit(s, 1, p2_fill.get(s, ()), p2_tr.get(s))
        # phase 3: s=6..31, both ch; remaining k/v jobs spread ~one per
        # two s-tiles so the PE stays ahead of the exp stream deep into
        # the tail; transposes of chunk c roughly at s=4c-2.
        p3_fill = {7: [jk(4)], 9: [jv(4)], 13: [jk(5)], 15: [jv(5)],
                   18: [jk(6)], 20: [jv(6)], 22: [jv(7)], 24: [jk(7)]}
        p3_tr = {8: 3, 12: 4, 17: 5, 21: 6, 26: 7}
        for s in range(6, NS):
            attn_unit(s, 0, p3_fill.get(s, ()), p3_tr.get(s))
            attn_unit(s, 1)
        while pend:
            flush_unit()


def _build():
    if _nc_cache:
        return _nc_cache[0]
    nc = bacc.Bacc("TRN2", target_bir_lowering=False, debug=False,
                   num_devices=NCORES)
    ap = {}
    ap["xT"] = nc.dram_tensor("xT", [DMODEL, T], BF, kind="ExternalInput")
    for nm in ("wk", "wq", "wv"):
        ap[nm] = nc.dram_tensor(nm, [DIM, DMODEL + 1], BF,
                                kind="ExternalInput")
    ap["outT"] = nc.dram_tensor("outT", [DIM, THALF], BF,
                                kind="ExternalOutput")
    ap["acc"] = nc.dram_tensor("acc", [DIM, THALF], BF,
                               kind="ExternalOutput")
    ap["e31"] = nc.dram_tensor("e31", [DIM, THALF], BF,
                               kind="ExternalOutput")

    with tile.TileContext(nc) as tc:
        _emit(nc, tc, ap)
    nc.compile()
    _nc_cache.append(nc)
    return nc


def _in_maps(x, W_qkv, b_qkv):
    """Host-side shard prep: de-interleave qkv weights, transpose x per batch."""
    # w<m>[p, (n, e)] = W_m[n*128 + p, e]; last col = bias
    wpk = {}
    for nm, j in (("wq", 0), ("wk", 1), ("wv", 2)):
        w = np.ascontiguousarray(W_qkv[:, j::3]) \
            .reshape(NDIN, 128, DIM).transpose(1, 0, 2).reshape(128, -1)
        wpk[nm] = np.concatenate([w, b_qkv[j::3][:, None]],
                                 axis=1).astype(BF16)

    maps = []
    for core in range(NCORES):
        b, half = divmod(core, 2)
        xTb = np.ascontiguousarray(x[b].T.astype(BF16))   # [1024, 4096]
        if half == 1:
            xTb = np.ascontiguousarray(
                np.concatenate([xTb[:, THALF:], xTb[:, :THALF]], axis=1))
        maps.append({"xT": xTb, **wpk})
    return maps


LAST_EXEC_NS = None
LAST_TRACE_PATH = None


def kernel(x, W_qkv, b_qkv):
    global LAST_EXEC_NS, LAST_TRACE_PATH
    import os
    x = np.asarray(x, dtype=np.float32)
    W_qkv = np.asarray(W_qkv, dtype=np.float32)
    b_qkv = np.asarray(b_qkv, dtype=np.float32)
    nc = _build()
    maps = _in_maps(x, W_qkv, b_qkv)
    trace = bool(os.environ.get("ATTN_TRACE"))
    res = bass_utils.run_bass_kernel_spmd(nc, maps, core_ids=list(range(NCORES)),
                                          trace=trace)
    if res.exec_time_ns:
        LAST_EXEC_NS = res.exec_time_ns
        if res.instructions_and_trace:
            LAST_TRACE_PATH = res.instructions_and_trace[1]
    out = np.empty((B, T, DIM), np.float32)
    for core in range(NCORES):
        b, half = divmod(core, 2)
        outT = res.results[core]["outT"].astype(np.float64)     # [128, 2048]
        acc = res.results[core]["acc"].astype(np.float64)       # [128, 2048]
        e31 = res.results[core]["e31"].astype(np.float64)       # [128, 2048]
        denom = acc.sum(axis=0) + e31.sum(axis=0)               # [2048]
        out[b, half * THALF:(half + 1) * THALF] = (outT / denom[None, :]).T
    return out


# revision 14
# speedup vs baseline: 1.0072x; 1.0072x over previous
"""Single-head attention (qkv-proj + softmax(QK^T)V) on 8 TRN2 NeuronCores.

Sharding: batch (4) x query-half (2) -> 8 shards. Each core computes full
k/v for its batch (duplicated across the 2 cores sharing a batch) and
attention for its 2048 query rows. For odd cores the host rotates the
sequence axis of x^T so the core's own query half occupies columns 0:2048;
k/v ordering over s is irrelevant (softmax sum + AV contraction are
permutation-invariant when k and v share the ordering).

Per-core device kernel (bf16 matmuls, fp32 PSUM accumulation). Work is
emitted as (s-tile, ch) UNITS of 1024 query columns each (ch = q-column
half): 2 scores matmuls -> one [128,1024] PSUM tile -> one Exp activation
(FD=1024, scale fused, no max subtraction -- scores are bounded ~8) -> 2
AV matmuls accumulating o_t[ch] [128,1024] PSUM over all 32 s-tiles. AV
for a unit is emitted 4 units late (pend depth 4) so TensorE never waits
on ScalarE's exp stream.

The unit split (vs whole 2048-col s-tiles) exists for the FRONT of the
pipeline: ch0 units need only q chunks 0-1, so the scores/exp stream
starts as soon as x cols 0:1024 have landed (~17us) instead of waiting
for all of qT (~25us). Unit order: (s=0..5, ch0), (s=0..5, ch1), then
(s, ch0),(s, ch1) for s=6..31. Projection jobs (8 accumulation matmuls +
DVE bias-copy each) fill the PE between units, deadline-placed with 2-3
s-tiles of slack against their consumers and the x^T DMA wave arrivals.

HAM warmup starts at ~6.8us off a GpSimd-memset tile (GpSimd's queue is
free at kernel start; DVE's isn't) so the PE clock gate is 8/8 and the
Exp ACT_TABLE is loaded before the first projection matmul issues.

Softmax denominators: VectorE ping-pong-accumulates acc[ch] += exp unit
(bf16 2x mode); the 128-partition reduction and the divide run on the
host in float64. The last unit per ch ships its exp tile raw (e31); the
host folds its column sums into the denominator. Outputs outT/acc/e31
drain per-ch so ch0's copies+DMAs overlap ch1's final AV matmuls.

PSUM budget: scores pool 2x[128,1024] (4 banks, shared with projection
jobs' [128,512] tiles) + two o_t accumulators (4 banks) = 8 banks.
"""

import numpy as np
import ml_dtypes

import concourse.bass as bass
import concourse.tile as tile
from concourse import bacc, mybir
from concourse import bass_utils

BF16 = ml_dtypes.bfloat16
F32 = mybir.dt.float32
BF = mybir.dt.bfloat16
AF = mybir.ActivationFunctionType

B = 4
T = 4096
DMODEL = 1024
DIM = 128
NCORES = 8
THALF = T // 2          # 2048 query rows per core
NDIN = DMODEL // 128    # 8 contraction tiles
NS = T // 128           # 32 key/value s-tiles
SCALE = float(DIM) ** -0.5

_nc_cache = []


def _emit(nc, tc, ap):
    P = 128
    from contextlib import ExitStack
    with ExitStack() as ctx:
        res = ctx.enter_context(tc.tile_pool(name="resident", bufs=1))

        # ---- batched input DMAs (few, large, multi-dim-AP transfers,
        # need-ordered on ONE HWDGE ring: the SDMA engines round-robin
        # between rings at packet granularity, so a second ring would
        # steal bandwidth from the first-needed transfer) ----
        xw = []
        woff = []

        def load_wave(cc, o, w, din0=0, ndin=NDIN):
            t_ = res.tile([P, ndin, w], BF, tag=f"xw{cc}", name=f"xw{cc}")
            src = ap["xT"].ap()[din0 * P:(din0 + ndin) * P, o:o + w] \
                .rearrange("(n p) w -> p n w", p=P)
            nc.sync.dma_start(t_[:], src)
            xw.append(t_)
            woff.append((o, w, din0, ndin))

        # weights in three per-tensor pieces so each lands just before its
        # first projection; wave 0 split by contraction half so the first
        # k projection can start on dins 0-3 while 4-7 are still arriving
        wtile = {}
        for nm in ("wk", "wq", "wv"):
            wtile[nm] = res.tile([P, NDIN * P + 1], BF, tag=nm, name=nm)
        nc.sync.dma_start(wtile["wk"][:], ap["wk"].ap())
        load_wave("0a", 0, 512, din0=0, ndin=4)
        load_wave("0b", 0, 512, din0=4, ndin=4)
        nc.sync.dma_start(wtile["wq"][:], ap["wq"].ap())
        load_wave("1", 512, 512)
        nc.sync.dma_start(wtile["wv"][:], ap["wv"].ap())
        load_wave("2a", 1024, 512)
        load_wave("2b", 1536, 512)
        load_wave("3", 2048, 1024)
        load_wave("4", 3072, 1024)

        w_sb = {nm: wtile[nm][:, 0:NDIN * P].rearrange(
            "p (n e) -> p n e", n=NDIN) for nm in wtile}
        bias_f = res.tile([P, 3], F32, tag="bias_f")
        for i, nm in enumerate(("wq", "wv", "wk")):
            nc.vector.tensor_copy(bias_f[:, i:i + 1],
                                  wtile[nm][:, NDIN * P:NDIN * P + 1])
        bias = {"bq": bias_f[:, 0:1], "bv": bias_f[:, 1:2],
                "bk": bias_f[:, 2:3]}

        def xchunk(d, c):
            """x^T [128, 512] slice for 512-col chunk c, din tile d."""
            o = c * 512
            for cc, (wo, w, din0, ndin) in enumerate(woff):
                if wo <= o < wo + w and din0 <= d < din0 + ndin:
                    return xw[cc][:, d - din0, o - wo:o - wo + 512]
            raise AssertionError

        kT = res.tile([P, T], BF, tag="kT")
        vT = res.tile([P, T], BF, tag="vT")
        qT = res.tile([P, THALF], BF, tag="qT")
        v_sb = res.tile([P, T], BF, tag="v_sb")
        # per-ch ping-pong accumulators for the softmax denominators
        accs = [[res.tile([P, 1024], BF, tag=f"acc{ch}{i}",
                          name=f"acc{ch}{i}") for i in range(2)]
                for ch in range(2)]
        outT_sb = res.tile([P, THALF], BF, tag="outT_sb")

        sc_ps = ctx.enter_context(
            tc.tile_pool(name="sc_ps", bufs=2, space="PSUM"))
        o_ps = ctx.enter_context(
            tc.tile_pool(name="o_ps", bufs=2, space="PSUM"))
        e_sb = ctx.enter_context(tc.tile_pool(name="e_sb", bufs=8))

        # HAM warmup: spin matmuls on a GpSimd-zeroed tile from ~6.8us
        # (GpSimd's queue is free right after the framework preamble;
        # DVE's memset would only land at ~7.8us). The 4096-cycle HAM
        # window then flips to 8/8 right as the first x^T wave lands, so
        # the real projections run at full clock from their first MM.
        warm = res.tile([P, 512], BF, tag="warm")
        nc.gpsimd.memset(warm[:], 0.0)
        # dummy 1-element Exp pulls the ~2.7us ACT_TABLE_LOAD into the
        # idle head window instead of stalling the first real exp
        ewarm = res.tile([P, 1], BF, tag="ewarm")
        nc.scalar.activation(ewarm[:], warm[:, 0:1], AF.Exp,
                             bias=0.0, scale=1.0)
        # one accumulation group -> no inter-matmul semaphores, so the PE
        # actually runs back-to-back and the HAM activity window fills
        wps = sc_ps.tile([P, 512], F32, tag="sc", name="wps")
        NWARM = 11
        for i in range(NWARM):
            nc.tensor.matmul(wps[:], warm[:, 0:P], warm[:],
                             start=(i == 0), stop=(i == NWARM - 1))

        def proj_job(c, dst, wnm, bnm):
            """One projection job: 512 cols of dst via 8 accumulating
            matmuls (PSUM tile borrowed from the scores pool) + DVE
            bias-add copy."""
            p = sc_ps.tile([P, 512], F32, tag="sc", name="pj")
            for din in range(NDIN):
                nc.tensor.matmul(
                    p[:], w_sb[wnm][:, din], xchunk(din, c),
                    start=(din == 0), stop=(din == NDIN - 1))
            nc.vector.tensor_scalar_add(
                dst[:, c * 512:(c + 1) * 512], p[:], bias[bnm])

        def v_transposes(tiles):
            # v natural tiles via DMA xbar transposes on the sync ring;
            # issued per s-tile (~1.2us issue each) spread one-ish per
            # unit so no flush ever waits on a 4-tile issue burst
            for s in tiles:
                nc.sync.dma_start_transpose(
                    v_sb[:, s * P:(s + 1) * P], vT[:, s * P:(s + 1) * P])

        o_t = [o_ps.tile([P, 1024], F32, tag="o", name=f"o_t{i}")
               for i in range(2)]
        pend = []
        nacc = [0, 0]

        def flush_unit():
            e, s, ch = pend.pop(0)
            vs = v_sb[:, s * P:(s + 1) * P]
            st, sp = (s == 0), (s == NS - 1)
            nc.tensor.matmul(o_t[ch][:, 0:512], vs, e[:, 0:512],
                             start=st, stop=sp)
            nc.tensor.matmul(o_t[ch][:, 512:1024], vs, e[:, 512:1024],
                             start=st, stop=sp)
            if s == NS - 1:
                # last exp unit ships raw; the host folds its column sums
                # into the denominator
                nc.sync.dma_start(
                    ap["e31"].ap()[:, ch * 1024:(ch + 1) * 1024], e[:])
                # o_t[ch] is now complete: drain it (copies split across
                # DVE and ScalarE; ch0's drain overlaps ch1's last AVs)
                for j in range(2):
                    lo = ch * 1024 + j * 512
                    piece = outT_sb[:, lo:lo + 512]
                    src = o_t[ch][:, j * 512:j * 512 + 512]
                    if j == 0:
                        nc.vector.tensor_copy(piece, src)
                    else:
                        nc.scalar.copy(piece, src)
                    nc.sync.dma_start(ap["outT"].ap()[:, lo:lo + 512], piece)
                return
            n = nacc[ch]
            dst = accs[ch][n % 2][:]
            if n == 0:
                nc.vector.tensor_copy(dst, e[:])
            else:
                nc.vector.tensor_add(dst, accs[ch][(n - 1) % 2][:], e[:])
            nacc[ch] = n + 1
            if s == NS - 2:
                # acc[ch] is final (sums s=0..30); its DMA is queued before
                # the e31/outT DMAs of this ch and the ring is FIFO
                nc.sync.dma_start(
                    ap["acc"].ap()[:, ch * 1024:(ch + 1) * 1024], dst)

        def attn_unit(s, ch, jobs=(), tr=None, spacer=False):
            # PE order: AV flush first (its exp landed 4 units ago --
            # guaranteed-ready work that absorbs the PSUM-slot recycle
            # wait of the projection job), then the projection (so its
            # DVE bias-add overlaps the scores matmuls instead of
            # stalling the NEXT unit's projection), then scores.
            if len(pend) >= 4:
                flush_unit()
            elif spacer:
                # pre-flush units have no ready AV work; burn two dummy
                # matmuls into o_t[1] (whose first real AV, start=True,
                # clears the bank) so the PE stays busy across the
                # previous projection's DVE bias-add turnaround
                for _ in range(2):
                    nc.tensor.matmul(o_t[1][:, 0:512], warm[:, 0:P],
                                     warm[:], start=True, stop=True)
            for job in jobs:
                proj_job(*job)
            ks = kT[:, s * P:(s + 1) * P]
            sc = sc_ps.tile([P, 1024], F32, tag="sc", name=f"sc{s}_{ch}")
            q0 = ch * 1024
            nc.tensor.matmul(sc[:, 0:512], ks, qT[:, q0:q0 + 512],
                             start=True, stop=True)
            nc.tensor.matmul(sc[:, 512:1024], ks, qT[:, q0 + 512:q0 + 1024],
                             start=True, stop=True)
            if tr is not None:
                v_transposes(tr)
            e = e_sb.tile([P, 1024], BF, tag="e", name=f"e{s}_{ch}")
            nc.scalar.activation(e[:], sc[:], AF.Exp, bias=0.0, scale=SCALE)
            pend.append((e, s, ch))

        # ---- emission ----
        # Up-front projections: k c0 (kT tiles 0-3), q c0, q c1 -- the
        # strict minimum before unit (0, ch0), so the exp stream starts
        # as soon as the w1 x^T wave lands.
        proj_job(0, kT, "wk", "bk")
        proj_job(0, qT, "wq", "bq")
        proj_job(1, qT, "wq", "bq")

        jk = lambda c: (c, kT, "wk", "bk")
        jv = lambda c: (c, vT, "wv", "bv")
        jq = lambda c: (c, qT, "wq", "bq")

        # Fillers spread ~uniformly (one job per 2-3 units) so the PE
        # stays ahead of the 1.1us/unit exp stream all the way into the
        # tail; each job lands 2+ s-tiles before its consumer (k c by
        # scores(4c, ch0), v c by its first tile transpose) and after
        # its x^T wave.  v-tile transposes go one per unit.
        # phase 1: (s=0..5, ch0); q c2/c3 land before phase 2 needs them.
        p1_fill = {1: [jv(0)], 2: [jk(1)], 3: [jv(1)], 4: [jq(2)],
                   5: [jq(3)]}
        p1_tr = {2: (0, 1), 3: (2, 3), 4: (4,), 5: (5,)}
        for s in range(6):
            attn_unit(s, 0, p1_fill.get(s, ()), p1_tr.get(s),
                      spacer=(s in (1, 2, 3)))
        # phase 2: (s=0..5, ch1)
        p2_fill = {0: [jk(2)], 1: [jv(2)], 3: [jk(3)], 5: [jv(3)]}
        p2_tr = {0: (6,), 1: (7,), 2: (8,), 3: (9,), 4: (10,), 5: (11,)}
        for s in range(6):
            attn_unit(s, 1, p2_fill.get(s, ()), p2_tr.get(s))
        # phase 3: s=6..31, both ch; v-tile s+6 transposes at (s, ch0),
        # 8 s-tiles ahead of its AV flush.
        p3_fill = {7: [jk(4)], 9: [jv(4)], 13: [jv(5)], 15: [jk(5)],
                   17: [jv(6)], 19: [jk(6)], 21: [jv(7)], 23: [jk(7)]}
        for s in range(6, NS):
            tr = (s + 6,) if s + 6 < NS else None
            attn_unit(s, 0, p3_fill.get(s, ()), tr)
            attn_unit(s, 1)
        while pend:
            flush_unit()


def _build():
    if _nc_cache:
        return _nc_cache[0]
    nc = bacc.Bacc("TRN2", target_bir_lowering=False, debug=False,
                   num_devices=NCORES)
    ap = {}
    ap["xT"] = nc.dram_tensor("xT", [DMODEL, T], BF, kind="ExternalInput")
    for nm in ("wk", "wq", "wv"):
        ap[nm] = nc.dram_tensor(nm, [DIM, DMODEL + 1], BF,
                                kind="ExternalInput")
    ap["outT"] = nc.dram_tensor("outT", [DIM, THALF], BF,
                                kind="ExternalOutput")
    ap["acc"] = nc.dram_tensor("acc", [DIM, THALF], BF,
                               kind="ExternalOutput")
    ap["e31"] = nc.dram_tensor("e31", [DIM, THALF], BF,
                               kind="ExternalOutput")

    with tile.TileContext(nc) as tc:
        _emit(nc, tc, ap)
    nc.compile()
    _nc_cache.append(nc)
    return nc


def _in_maps(x, W_qkv, b_qkv):
    """Host-side shard prep: de-interleave qkv weights, transpose x per batch."""
    # w<m>[p, (n, e)] = W_m[n*128 + p, e]; last col = bias
    wpk = {}
    for nm, j in (("wq", 0), ("wk", 1), ("wv", 2)):
        w = np.ascontiguousarray(W_qkv[:, j::3]) \
            .reshape(NDIN, 128, DIM).transpose(1, 0, 2).reshape(128, -1)
        wpk[nm] = np.concatenate([w, b_qkv[j::3][:, None]],
                                 axis=1).astype(BF16)

    maps = []
    for core in range(NCORES):
        b, half = divmod(core, 2)
        xTb = np.ascontiguousarray(x[b].T.astype(BF16))   # [1024, 4096]
        if half == 1:
            xTb = np.ascontiguousarray(
                np.concatenate([xTb[:, THALF:], xTb[:, :THALF]], axis=1))
        maps.append({"xT": xTb, **wpk})
    return maps


LAST_EXEC_NS = None
LAST_TRACE_PATH = None


def kernel(x, W_qkv, b_qkv):
    global LAST_EXEC_NS, LAST_TRACE_PATH
    import os
    x = np.asarray(x, dtype=np.float32)
    W_qkv = np.asarray(W_qkv, dtype=np.float32)
    b_qkv = np.asarray(b_qkv, dtype=np.float32)
    nc = _build()
    maps = _in_maps(x, W_qkv, b_qkv)
    trace = bool(os.environ.get("ATTN_TRACE"))
    res = bass_utils.run_bass_kernel_spmd(nc, maps, core_ids=list(range(NCORES)),
                                          trace=trace)
    if res.exec_time_ns:
        LAST_EXEC_NS = res.exec_time_ns
        if res.instructions_and_trace:
            LAST_TRACE_PATH = res.instructions_and_trace[1]
    out = np.empty((B, T, DIM), np.float32)
    for core in range(NCORES):
        b, half = divmod(core, 2)
        outT = res.results[core]["outT"].astype(np.float64)     # [128, 2048]
        acc = res.results[core]["acc"].astype(np.float64)       # [128, 2048]
        e31 = res.results[core]["e31"].astype(np.float64)       # [128, 2048]
        denom = acc.sum(axis=0) + e31.sum(axis=0)               # [2048]
        out[b, half * THALF:(half + 1) * THALF] = (outT / denom[None, :]).T
    return out
